# revision 28
# baseline (speedup 1.0000x reference)
"""Multi-head attention Trainium2 kernel (8 NeuronCores, SPMD).

Problem: nn_MultiHeadAttention (B=2, S=2048, D=768, H=12, d_k=64), f32 I/O.

Sharding: 24 (batch, head) pairs -> 8 cores x 3 heads. Core c handles
batch b = c // 4 and heads [3*(c%4), 3*(c%4)+3). Each core computes the
Q/K/V projections for its 3 heads, full-sequence attention, and its
partial contribution to the output projection. A 4-core ReduceScatter
(cores of the same batch) sums the partials and leaves each core with a
distinct 512-row slice of the batch output; the host concatenates.

v3 schedule: the kernel is ACT-bound (softmax exp = 98304 elem/partition
= ~82us at 1.2GHz), so everything is a software pipeline that keeps the
Activation engine streaming exps. HW facts driving the design (measured
via microbench.py): adjacent row-tile pairs (tile_position (0,0)/(64,0),
K=64) run at ~96 ns/MM vs 416 ns serial; tile-mode switches cost ~200 ns
so the PE stream is mode-grouped; K=128 serial matmuls run at ~165 ns.
  - heads 0,1 projected stacked on partitions 0..127; head 2's q/k are
    duplicated onto partitions 64..127 so its score matmuls pair too
  - every score round is one [128,2,512] PSUM tile = one kv chunk for
    two row-tile positions + a single 1024-elem exp; the round ring is
    double-buffered so ACT never waits on PE
  - PV / out-proj / projection matmuls are K=128 (128,128)-mode and are
    inserted between score rounds in <=2us mode-pure chunks
  - softmax reciprocal: DVE recip -> gpsimd partition_broadcast -> DVE
    multiply (no DRAM bounce)
  - output projection + ReduceScatter run per 512-row q-block inside
    the pipeline; bias enters via a ones-row appended to outT
"""

import numpy as np
import ml_dtypes

B = 2
S = 2048
D = 768
H = 12
DK = 64
HPC = 3           # heads per core
HD = HPC * DK     # 192 head-feature columns per core
NCORES = 8
GROUP = 4         # cores per batch (reduce-scatter group)
QS = S // GROUP   # 512 output rows per core

_compiled = None


def _build(reps=1, collective=True):
    """Build the SPMD program. reps>1 emits the whole pipeline N times
    back-to-back (same inputs/outputs) — used only for timing, where
    (T_reps - T_1)/(reps-1) cancels the per-dispatch overhead.
    collective=False drops the final ReduceScatter (for TimelineSim)."""
    import concourse.mybir as mybir
    import concourse.tile as tile
    from concourse import bacc
    from concourse.bass import ts
    import concourse.bass as bass

    bf16 = mybir.dt.bfloat16
    f32 = mybir.dt.float32

    nc = bacc.Bacc(num_devices=NCORES)

    qt = nc.dram_tensor("qt", [D, S], bf16, kind="ExternalInput")
    kt = nc.dram_tensor("kt", [D, S], bf16, kind="ExternalInput")
    vt = nc.dram_tensor("vt", [D, S], bf16, kind="ExternalInput")
    wq = nc.dram_tensor("wq", [D, HD], bf16, kind="ExternalInput")
    wk = nc.dram_tensor("wk", [D, HD], bf16, kind="ExternalInput")
    wv = nc.dram_tensor("wv", [D, HD], bf16, kind="ExternalInput")
    wo = nc.dram_tensor("wo", [HD + 1, D], bf16, kind="ExternalInput")
    bq = nc.dram_tensor("bq", [HD, 1], f32, kind="ExternalInput")
    bk = nc.dram_tensor("bk", [HD, 1], f32, kind="ExternalInput")
    bv = nc.dram_tensor("bv", [1, HD], f32, kind="ExternalInput")
    out_ext = nc.dram_tensor("out", [QS, D], bf16, kind="ExternalOutput")
    out_part = nc.dram_tensor("out_part", [S, D], bf16)
    out_rs = nc.dram_tensor("out_rs", [QS, D], bf16)

    RGROUPS = [list(range(g * GROUP, (g + 1) * GROUP))
               for g in range(NCORES // GROUP)]
    NC_ = D // 128      # 6 contraction chunks for the projections
    NKC = S // 128      # 16 kv chunks
    NQB = S // 512      # 4 q blocks
    VW = DK + 2         # 66-wide per-head V block: 64 dims + ones col + pad
    SCALE = float(1.0 / np.sqrt(DK))

    import contextlib

    with tile.TileContext(nc) as tc:
      with (tc.For_i(0, reps, 1) if reps > 1 else contextlib.nullcontext()):
       with contextlib.ExitStack() as ctx:
        consts = ctx.enter_context(tc.tile_pool(name="consts", bufs=1))
        acts = ctx.enter_context(tc.tile_pool(name="acts", bufs=1))

        # ---- load inputs, in consumption order ----
        # Two HWDGE queues (SP + Pool/gpsimd) so the ACT engine is never
        # burdened with DMA dispatch.
        dmae = [nc.sync, nc.gpsimd]
        ins_sb, w_sb, bias_sb = {}, {}, {}

        def load_w(name, t):
            sb = consts.tile([128, NC_, HD], bf16, tag=name)
            nc.gpsimd.dma_start(
                out=sb, in_=t[:, :].rearrange("(c p) n -> p c n", p=128))
            w_sb[name] = sb

        def load_bias(name, t):
            b0 = consts.tile([128, 1], f32, tag=name + "0")
            nc.sync.dma_start(out=b0, in_=t[0:128, :])
            b1 = consts.tile([HD - 128, 1], f32, tag=name + "1")
            nc.sync.dma_start(out=b1, in_=t[128:HD, :])
            bias_sb[name] = (b0, b1)

        def load_in(name, t, di=[0]):
            sb = consts.tile([128, NC_, S], bf16, tag=name)
            for c in range(NC_):
                dmae[di[0] % 2].dma_start(
                    out=sb[:, c, :], in_=t[c * 128:(c + 1) * 128, :])
                di[0] += 1
            ins_sb[name] = sb

        load_w("wk", wk)
        load_bias("bk", bk)
        load_in("kt", kt)
        load_w("wq", wq)
        load_bias("bq", bq)
        # qt: first 512 columns (q-block 0) first so scores start early
        qt_sb = consts.tile([128, NC_, S], bf16, tag="qt")
        for c in range(NC_):
            dmae[c % 2].dma_start(
                out=qt_sb[:, c, 0:512], in_=qt[c * 128:(c + 1) * 128, 0:512])
        for c in range(NC_):
            dmae[c % 2].dma_start(
                out=qt_sb[:, c, 512:S], in_=qt[c * 128:(c + 1) * 128, 512:S])
        ins_sb["qt"] = qt_sb
        load_w("wv", wv)
        bv_bc = consts.tile([128, HD], f32, tag="bv")
        nc.sync.dma_start(
            out=bv_bc,
            in_=bass.AP(tensor=bv[:, :].tensor, offset=bv[:, :].offset,
                        ap=[[0, 128]] + bv[:, :].ap[1:]))
        load_in("vt", vt)
        wo0 = consts.tile([128, D], bf16, tag="wo0")
        nc.gpsimd.dma_start(out=wo0, in_=wo[0:128, :])
        wo1 = consts.tile([HD + 1 - 128, D], bf16, tag="wo1")
        nc.gpsimd.dma_start(out=wo1, in_=wo[128:HD + 1, :])
        # Touch the exp table early so ACT's table DMA overlaps the loads.
        warm = consts.tile([1, 1], f32, tag="warm")
        nc.vector.memset(warm, 0.0)
        nc.scalar.activation(out=warm, in_=warm,
                             func=mybir.ActivationFunctionType.Exp)

        # ---- persistent SBUF activation tiles ----
        # heads 0,1 stacked [128, S]; head 2 as two [64, S] tiles
        qT0 = acts.tile([128, S], bf16, tag="qT0")
        kT0 = acts.tile([128, S], bf16, tag="kT0")
        qT1 = acts.tile([128, S], bf16, tag="qT1")
        kT1 = acts.tile([128, S], bf16, tag="kT1")
        qk0 = {"q": qT0, "k": kT0}
        qk1 = {"q": qT1, "k": kT1}
        outT0 = acts.tile([128, S], bf16, tag="outT0")
        outT1 = acts.tile([DK + 1, S], bf16, tag="outT1")
        nc.vector.memset(outT1[DK:DK + 1, :], 1.0)
        v_sb = acts.tile([128, NKC, HPC * VW], bf16, tag="v")
        for h in range(HPC):
            nc.vector.memset(v_sb[:, :, h * VW + DK:h * VW + DK + 1], 1.0)

        # PSUM budget (8 banks of [128,512]f32):
        #  sc: [128,2,512] x3 = 6 banks (score rounds: one kv chunk per
        #      row-tile position; depth 3 so ACT never waits on the sem
        #      round-trip — HW rounds run ~640ns at depth>=3 vs 911 at 2)
        #  pw: [128,512] x2 = 2 banks (q/g1/V projections, PV
        #      accumulators, out-proj tiles)
        sc_pool = ctx.enter_context(
            tc.tile_pool(name="sc_psum", bufs=3, space="PSUM"))
        pw_pool = ctx.enter_context(
            tc.tile_pool(name="pw_psum", bufs=2, space="PSUM"))
        sm_pool = ctx.enter_context(tc.tile_pool(name="sm", bufs=1))
        nrm_pool = ctx.enter_context(tc.tile_pool(name="nrm", bufs=4))
        fo_pool = ctx.enter_context(tc.tile_pool(name="fo", bufs=2))

        e01_t = {}       # qb -> expt tile h0/h1: [128, NKC, 2, 512]
        e2_t = {}        # qb -> expt tile h2:    [128, NKC, 512]
        pvps = {}        # h -> current PV PSUM tile

        # ---- k/q g0 projection for one 512-col block into psum ----
        def emit_proj_block(ps, name, qb):
            x_sb = ins_sb[name + "t"]
            for c in range(NC_):
                nc.tensor.matmul(
                    ps, lhsT=w_sb["w" + name][:, c, 0:128],
                    rhs=x_sb[:, c, ts(qb, 512)],
                    start=(c == 0), stop=(c == NC_ - 1))

        def evac_proj(ps, name, qb):
            nc.vector.tensor_scalar_add(
                out=qk0[name][:, ts(qb, 512)], in0=ps,
                scalar1=bias_sb["b" + name][0])

        # ---- units: closures emitted by the pipeline zipper ----
        def u_qproj(qb):
            def emit():
                ps = pw_pool.tile([128, 512], f32, tag="pw", name="pw")
                emit_proj_block(ps, "q", qb)
                evac_proj(ps, "q", qb)
            return emit

        def u_g1proj(qb):
            # head 2 q/k: M=64 pair on PE col groups 0-1 / 2-3
            def emit():
                ps = pw_pool.tile([128, 512], f32, tag="pw", name="pw")
                for c in range(NC_):
                    nc.tensor.matmul(
                        ps[0:64, :], lhsT=w_sb["wq"][:, c, 128:192],
                        rhs=ins_sb["qt"][:, c, ts(qb, 512)],
                        start=(c == 0), stop=(c == NC_ - 1),
                        tile_position=(0, 0))
                    nc.tensor.matmul(
                        ps[64:128, :], lhsT=w_sb["wk"][:, c, 128:192],
                        rhs=ins_sb["kt"][:, c, ts(qb, 512)],
                        start=(c == 0), stop=(c == NC_ - 1),
                        tile_position=(0, 64))
                nc.vector.tensor_scalar_add(
                    out=qk1["q"][0:64, ts(qb, 512)], in0=ps[0:64, :],
                    scalar1=bias_sb["bq"][1])
                nc.vector.tensor_scalar_add(
                    out=qk1["k"][0:64, ts(qb, 512)], in0=ps[64:128, :],
                    scalar1=bias_sb["bk"][1])
            return emit

        def u_g1dup():
            # duplicate head-2 q/k onto partitions 64..127 so its score
            # matmuls can run as row-tile pairs too
            def emit():
                nc.sync.dma_start(out=qk1["q"][64:128, :],
                                  in_=qk1["q"][0:64, :])
                nc.sync.dma_start(out=qk1["k"][64:128, :],
                                  in_=qk1["k"][0:64, :])
            return emit

        def u_vproj(st):
            def emit():
                ps = pw_pool.tile([128, 512], f32, tag="pw", name="pw")
                for c in range(NC_):
                    nc.tensor.matmul(
                        ps[:, 0:HD], lhsT=ins_sb["vt"][:, c, ts(st, 128)],
                        rhs=w_sb["wv"][:, c, :],
                        start=(c == 0), stop=(c == NC_ - 1))
                for h in range(HPC):
                    nc.vector.tensor_add(
                        v_sb[:, st, h * VW:h * VW + DK],
                        ps[:, ts(h, 64)], bv_bc[:, ts(h, 64)])
            return emit

        def u_sc01(qb, c):
            # heads 0+1, kv chunk c, as an adjacent row-tile pair into one
            # [128,2,512] tile; one 1024-elem exp
            def emit():
                if c == 0:
                    e01_t[qb] = sm_pool.tile([128, NKC, 2, 512], bf16,
                                             tag="e01", name="e01")
                ps = sc_pool.tile([128, 2, 512], f32, tag="sc", name="sc")
                for h in (0, 1):
                    nc.tensor.matmul(
                        ps[:, h, :],
                        lhsT=qk0["k"][ts(h, 64), ts(c, 128)],
                        rhs=qk0["q"][ts(h, 64), ts(qb, 512)],
                        start=True, stop=True)
                nc.scalar.activation(
                    out=e01_t[qb][:, c, :, :], in_=ps[:, 0:2, :],
                    func=mybir.ActivationFunctionType.Exp, scale=SCALE)
            return emit

        def u_sc2(qb, p):
            # head 2, kv chunks {2p, 2p+1} as a row-tile pair (second
            # operand pair comes from the duplicated partitions 64..127)
            def emit():
                if p == 0:
                    e2_t[qb] = sm_pool.tile([128, NKC, 512], bf16,
                                            tag="e2", name="e2")
                ps = sc_pool.tile([128, 2, 512], f32, tag="sc", name="sc")
                for j in (0, 1):
                    nc.tensor.matmul(
                        ps[:, j, :],
                        lhsT=qk1["k"][ts(j, 64), ts(2 * p + j, 128)],
                        rhs=qk1["q"][ts(j, 64), ts(qb, 512)],
                        start=True, stop=True)
                nc.scalar.activation(
                    out=e2_t[qb][:, 2 * p:2 * p + 2, :], in_=ps[:, 0:2, :],
                    func=mybir.ActivationFunctionType.Exp, scale=SCALE)
            return emit

        def u_pv(h, qb, half):
            # PV accumulation, kv chunks [8*half, 8*half+8)
            def emit():
                if half == 0:
                    pvps[h] = pw_pool.tile([128, 512], f32, tag="pw",
                                           name="pw")
                ps = pvps[h]
                for kc in range(8 * half, 8 * half + 8):
                    rhs = (e01_t[qb][:, kc, h, :] if h < 2
                           else e2_t[qb][:, kc, :])
                    nc.tensor.matmul(
                        ps[0:DK + 1, :],
                        lhsT=v_sb[:, kc, h * VW:h * VW + DK + 1],
                        rhs=rhs,
                        start=(kc == 0), stop=(kc == NKC - 1))
            return emit

        def u_norm(h, qb):
            # reciprocal of the ones-row -> SBUF, partition-broadcast on
            # the idle gpsimd engine, then normalize on DVE.
            def emit():
                ps = pvps[h]
                recip = nrm_pool.tile([1, 512], bf16, tag="recip",
                                      name="recip")
                with nc.allow_low_precision(reason="softmax recip in bf16"):
                    nc.vector.reciprocal(recip, ps[DK:DK + 1, :])
                rbc = nrm_pool.tile([64, 512], bf16, tag="rbc", name="rbc")
                nc.gpsimd.partition_broadcast(rbc, recip)
                dst = (outT0[ts(h, 64), ts(qb, 512)] if h < 2
                       else outT1[0:64, ts(qb, 512)])
                nc.vector.tensor_mul(dst, ps[0:DK, :], rbc)
            return emit

        ot_tile = {}

        def u_oproj(qb, sq):
            # output projection for q sub-tile qb*4+sq, N split 512+256
            def emit():
                if sq == 0:
                    ot_tile[qb] = fo_pool.tile([128, 4, D], bf16, tag="ot",
                                               name="ot")
                qt_ = qb * 4 + sq
                for noff, nsz in ((0, 512), (512, 256)):
                    ps = pw_pool.tile([128, 512], f32, tag="pw", name="pw")
                    nc.tensor.matmul(
                        ps[:, 0:nsz], lhsT=outT0[:, ts(qt_, 128)],
                        rhs=wo0[:, noff:noff + nsz], start=True, stop=False)
                    nc.tensor.matmul(
                        ps[:, 0:nsz], lhsT=outT1[:, ts(qt_, 128)],
                        rhs=wo1[:, noff:noff + nsz], start=False, stop=True)
                    nc.vector.tensor_copy(
                        out=ot_tile[qb][:, sq, noff:noff + nsz],
                        in_=ps[:, 0:nsz])
            return emit

        def u_out(qb):
            def emit():
                nc.sync.dma_start(
                    out=out_part[:, :].rearrange(
                        "(g t p) d -> g p t d", p=128, t=4)[qb],
                    in_=ot_tile[qb])
                if collective:
                    nc.gpsimd.collective_compute(
                        "ReduceScatter", mybir.AluOpType.add,
                        replica_groups=RGROUPS,
                        ins=[out_part[ts(qb, 512), :]],
                        outs=[out_rs[ts(qb, 128), :]])
                nc.sync.dma_start(out=out_ext[ts(qb, 128), :],
                                  in_=(out_rs if collective
                                       else out_part)[ts(qb, 128), :])
            return emit

        def zip_emit(primary, secondary):
            """Interleave unit lists: primary paces (ACT-gated score
            rounds), secondary fills PE slack in mode-pure chunks."""
            n = max(len(primary), len(secondary))
            for i in range(n):
                if i < len(primary):
                    primary[i]()
                if i < len(secondary):
                    secondary[i]()

        # ---- prologue: k projection (both k-qb pairs share one sc tile
        # via the 2-chunk slots), then q block 0 ----
        for pair in (0, 1):
            ps = sc_pool.tile([128, 2, 512], f32, tag="sc", name="sc")
            for j in (0, 1):
                qb_ = 2 * pair + j
                emit_proj_block(ps[:, j, :], "k", qb_)
                evac_proj(ps[:, j, :], "k", qb_)
        ps = pw_pool.tile([128, 512], f32, tag="pw", name="pw")
        emit_proj_block(ps, "q", 0)
        evac_proj(ps, "q", 0)

        # ---- pipeline over q blocks ----
        # iteration qb emits its own score rounds (h2 first, then h0/h1;
        # qb0: h0/h1 first since head 2 projection happens inside qb0's
        # secondary list), with the previous block's PV/norm/out-proj
        # (and qb's own h2 PV) zipped between rounds in mode-pure chunks.
        for it in range(NQB + 1):
            qb, pq = it, it - 1
            if it == 0:
                rounds = ([u_sc01(0, c) for c in range(NKC)] +
                          [u_sc2(0, p) for p in range(NKC // 2)])
                sec = ([u_qproj(1), u_qproj(2), u_qproj(3)] +
                       [u_g1proj(qb_) for qb_ in range(NQB)] +
                       [u_g1dup()] +
                       [u_vproj(st) for st in range(NKC)])
            elif it < NQB:
                rounds = ([u_sc2(qb, p) for p in range(NKC // 2)] +
                          [u_sc01(qb, c) for c in range(NKC)])
                # pv2(pq) must come first: the new h2 exps reuse its tile
                sec = [u_pv(2, pq, 0), u_pv(2, pq, 1), u_norm(2, pq),
                       u_pv(0, pq, 0), u_pv(0, pq, 1), u_norm(0, pq),
                       u_pv(1, pq, 0), u_pv(1, pq, 1), u_norm(1, pq),
                       u_oproj(pq, 0), u_oproj(pq, 1),
                       u_oproj(pq, 2), u_oproj(pq, 3), u_out(pq)]
                if it == NQB - 1:
                    # drain qb3's h2 inside its own iteration to shorten
                    # the epilogue
                    sec += [u_pv(2, qb, 0), u_pv(2, qb, 1), u_norm(2, qb)]
            else:
                rounds = []
                sec = [u_pv(0, pq, 0), u_pv(0, pq, 1), u_norm(0, pq),
                       u_pv(1, pq, 0), u_pv(1, pq, 1), u_norm(1, pq),
                       u_oproj(pq, 0), u_oproj(pq, 1),
                       u_oproj(pq, 2), u_oproj(pq, 3), u_out(pq)]
            zip_emit(rounds, sec)

    nc.compile()
    return nc


def _get_compiled():
    global _compiled
    if _compiled is None:
        _compiled = _build()
    return _compiled


def make_in_maps(q, k, v, Wq, bq, Wk, bk, Wv, bv, Wo, bo):
    bf = ml_dtypes.bfloat16
    in_maps = []
    for c in range(NCORES):
        b = c // GROUP
        g = c % GROUP
        cols = slice(g * HD, (g + 1) * HD)   # head-feature columns
        wo_aug = np.empty((HD + 1, D), np.float32)
        wo_aug[:HD] = Wo.T[cols.start:cols.stop, :]
        wo_aug[HD] = bo / GROUP              # summed GROUP times by the RS
        in_maps.append({
            "qt": np.ascontiguousarray(q[b].T).astype(bf),
            "kt": np.ascontiguousarray(k[b].T).astype(bf),
            "vt": np.ascontiguousarray(v[b].T).astype(bf),
            "wq": np.ascontiguousarray(Wq.T[:, cols]).astype(bf),
            "wk": np.ascontiguousarray(Wk.T[:, cols]).astype(bf),
            "wv": np.ascontiguousarray(Wv.T[:, cols]).astype(bf),
            "wo": wo_aug.astype(bf),
            "bq": np.ascontiguousarray(bq[cols].reshape(HD, 1)).astype(np.float32),
            "bk": np.ascontiguousarray(bk[cols].reshape(HD, 1)).astype(np.float32),
            "bv": np.ascontiguousarray(bv[cols].reshape(1, HD)).astype(np.float32),
        })
    return in_maps


def kernel(q, k, v, Wq, bq, Wk, bk, Wv, bv, Wo, bo):
    from concourse.bass_utils import run_bass_kernel_spmd

    q = np.asarray(q, np.float32)
    k = np.asarray(k, np.float32)
    v = np.asarray(v, np.float32)
    nc = _get_compiled()
    in_maps = make_in_maps(q, k, v,
                           np.asarray(Wq, np.float32), np.asarray(bq, np.float32),
                           np.asarray(Wk, np.float32), np.asarray(bk, np.float32),
                           np.asarray(Wv, np.float32), np.asarray(bv, np.float32),
                           np.asarray(Wo, np.float32), np.asarray(bo, np.float32))
    res = run_bass_kernel_spmd(nc, in_maps, list(range(NCORES))).results
    out = np.empty((B, S, D), np.float32)
    for c in range(NCORES):
        b = c // GROUP
        j = c % GROUP
        # chunked reduce-scatter: chunk g of core (b, j) holds batch-b
        # rows [512*g + 128*j, 512*g + 128*j + 128)
        chunks = res[c]["out"].reshape(GROUP, 128, D)
        for g in range(GROUP):
            out[b, 512 * g + 128 * j:512 * g + 128 * j + 128, :] = chunks[g]
    return out


# revision 29
# speedup vs baseline: 1.1126x; 1.1126x over previous
"""Multi-head attention Trainium2 kernel (8 NeuronCores, SPMD).

Problem: nn_MultiHeadAttention (B=2, S=2048, D=768, H=12, d_k=64), f32 I/O.

Sharding: 24 (batch, head) pairs -> 8 cores x 3 heads. Core c handles
batch b = c // 4 and heads [3*(c%4), 3*(c%4)+3). Each core computes the
Q/K/V projections for its 3 heads, full-sequence attention, and its
partial contribution to the output projection. A 4-core ReduceScatter
(cores of the same batch) sums the partials and leaves each core with a
distinct 512-row slice of the batch output; the host concatenates.

v3 schedule: the kernel is ACT-bound (softmax exp = 98304 elem/partition
= ~82us at 1.2GHz), so everything is a software pipeline that keeps the
Activation engine streaming exps. HW facts driving the design (measured
via microbench.py): adjacent row-tile pairs (tile_position (0,0)/(64,0),
K=64) run at ~96 ns/MM vs 416 ns serial; tile-mode switches cost ~200 ns
so the PE stream is mode-grouped; K=128 serial matmuls run at ~165 ns.
  - heads 0,1 projected stacked on partitions 0..127; head 2's q/k are
    duplicated onto partitions 64..127 so its score matmuls pair too
  - every score round is one [128,2,512] PSUM tile = one kv chunk for
    two row-tile positions + a single 1024-elem exp; the round ring is
    double-buffered so ACT never waits on PE
  - PV / out-proj / projection matmuls are K=128 (128,128)-mode and are
    inserted between score rounds in <=2us mode-pure chunks
  - softmax reciprocal: DVE recip -> gpsimd partition_broadcast -> DVE
    multiply (no DRAM bounce)
  - output projection + ReduceScatter run per 512-row q-block inside
    the pipeline; bias enters via a ones-row appended to outT
"""

import numpy as np
import ml_dtypes

B = 2
S = 2048
D = 768
H = 12
DK = 64
HPC = 3           # heads per core
HD = HPC * DK     # 192 head-feature columns per core
NCORES = 8
GROUP = 4         # cores per batch (reduce-scatter group)
QS = S // GROUP   # 512 output rows per core

_compiled = None


def _build(reps=1, collective=True):
    """Build the SPMD program. reps>1 emits the whole pipeline N times
    back-to-back (same inputs/outputs) — used only for timing, where
    (T_reps - T_1)/(reps-1) cancels the per-dispatch overhead.
    collective=False drops the final ReduceScatter (for TimelineSim)."""
    import concourse.mybir as mybir
    import concourse.tile as tile
    from concourse import bacc
    from concourse.bass import ts
    import concourse.bass as bass

    bf16 = mybir.dt.bfloat16
    f32 = mybir.dt.float32

    nc = bacc.Bacc(num_devices=NCORES)

    qt = nc.dram_tensor("qt", [D, S], bf16, kind="ExternalInput")
    kt = nc.dram_tensor("kt", [D, S], bf16, kind="ExternalInput")
    vt = nc.dram_tensor("vt", [D, S], bf16, kind="ExternalInput")
    wq = nc.dram_tensor("wq", [D, HD], bf16, kind="ExternalInput")
    wk = nc.dram_tensor("wk", [D, HD], bf16, kind="ExternalInput")
    wv = nc.dram_tensor("wv", [D, HD], bf16, kind="ExternalInput")
    wo = nc.dram_tensor("wo", [HD + 1, D], bf16, kind="ExternalInput")
    bq = nc.dram_tensor("bq", [HD, 1], f32, kind="ExternalInput")
    bk = nc.dram_tensor("bk", [HD, 1], f32, kind="ExternalInput")
    bv = nc.dram_tensor("bv", [1, HD], f32, kind="ExternalInput")
    out_ext = nc.dram_tensor("out", [QS, D], bf16, kind="ExternalOutput")
    out_part = nc.dram_tensor("out_part", [S, D], bf16)
    out_rs = nc.dram_tensor("out_rs", [QS, D], bf16)

    RGROUPS = [list(range(g * GROUP, (g + 1) * GROUP))
               for g in range(NCORES // GROUP)]
    NC_ = D // 128      # 6 contraction chunks for the projections
    NKC = S // 128      # 16 kv chunks
    NQB = S // 512      # 4 q blocks
    VW = DK + 2         # 66-wide per-head V block: 64 dims + ones col + pad
    SCALE = float(1.0 / np.sqrt(DK))

    import contextlib

    with tile.TileContext(nc) as tc:
      with (tc.For_i(0, reps, 1) if reps > 1 else contextlib.nullcontext()):
       with contextlib.ExitStack() as ctx:
        consts = ctx.enter_context(tc.tile_pool(name="consts", bufs=1))
        acts = ctx.enter_context(tc.tile_pool(name="acts", bufs=1))

        # ---- load inputs, in consumption order ----
        # Two HWDGE queues (SP + Pool/gpsimd) so the ACT engine is never
        # burdened with DMA dispatch.
        dmae = [nc.sync, nc.gpsimd]
        ins_sb, w_sb, bias_sb = {}, {}, {}

        def load_w(name, t):
            sb = consts.tile([128, NC_, HD], bf16, tag=name)
            nc.gpsimd.dma_start(
                out=sb, in_=t[:, :].rearrange("(c p) n -> p c n", p=128))
            w_sb[name] = sb

        def load_bias(name, t):
            b0 = consts.tile([128, 1], f32, tag=name + "0")
            nc.sync.dma_start(out=b0, in_=t[0:128, :])
            b1 = consts.tile([HD - 128, 1], f32, tag=name + "1")
            nc.sync.dma_start(out=b1, in_=t[128:HD, :])
            bias_sb[name] = (b0, b1)

        def load_in(name, t, di=[0]):
            sb = consts.tile([128, NC_, S], bf16, tag=name)
            for c in range(NC_):
                dmae[di[0] % 2].dma_start(
                    out=sb[:, c, :], in_=t[c * 128:(c + 1) * 128, :])
                di[0] += 1
            ins_sb[name] = sb

        load_w("wk", wk)
        load_bias("bk", bk)
        load_in("kt", kt)
        load_w("wq", wq)
        load_bias("bq", bq)
        # qt: first 512 columns (q-block 0) first so scores start early
        qt_sb = consts.tile([128, NC_, S], bf16, tag="qt")
        for c in range(NC_):
            dmae[c % 2].dma_start(
                out=qt_sb[:, c, 0:512], in_=qt[c * 128:(c + 1) * 128, 0:512])
        for c in range(NC_):
            dmae[c % 2].dma_start(
                out=qt_sb[:, c, 512:S], in_=qt[c * 128:(c + 1) * 128, 512:S])
        ins_sb["qt"] = qt_sb
        load_w("wv", wv)
        bv_bc = consts.tile([128, HD], f32, tag="bv")
        nc.sync.dma_start(
            out=bv_bc,
            in_=bass.AP(tensor=bv[:, :].tensor, offset=bv[:, :].offset,
                        ap=[[0, 128]] + bv[:, :].ap[1:]))
        load_in("vt", vt)
        wo0 = consts.tile([128, D], bf16, tag="wo0")
        nc.gpsimd.dma_start(out=wo0, in_=wo[0:128, :])
        wo1 = consts.tile([HD + 1 - 128, D], bf16, tag="wo1")
        nc.gpsimd.dma_start(out=wo1, in_=wo[128:HD + 1, :])
        # Touch the exp table early so ACT's table DMA overlaps the loads.
        warm = consts.tile([1, 1], f32, tag="warm")
        nc.vector.memset(warm, 0.0)
        nc.scalar.activation(out=warm, in_=warm,
                             func=mybir.ActivationFunctionType.Exp)

        # ---- persistent SBUF activation tiles ----
        # heads 0,1 stacked [128, S]; head 2 as two [64, S] tiles
        qT0 = acts.tile([128, S], bf16, tag="qT0")
        kT0 = acts.tile([128, S], bf16, tag="kT0")
        qT1 = acts.tile([128, S], bf16, tag="qT1")
        kT1 = acts.tile([128, S], bf16, tag="kT1")
        qk0 = {"q": qT0, "k": kT0}
        qk1 = {"q": qT1, "k": kT1}
        outT0 = acts.tile([128, S], bf16, tag="outT0")
        outT1 = acts.tile([DK + 1, S], bf16, tag="outT1")
        nc.vector.memset(outT1[DK:DK + 1, :], 1.0)
        v_sb = acts.tile([128, NKC, HPC * VW], bf16, tag="v")
        for h in range(HPC):
            nc.vector.memset(v_sb[:, :, h * VW + DK:h * VW + DK + 1], 1.0)

        # PSUM budget (8 banks of [128,512]f32):
        #  sc: [128,2,512] x3 = 6 banks (score rounds: one kv chunk per
        #      row-tile position; depth 3 so ACT never waits on the sem
        #      round-trip — HW rounds run ~640ns at depth>=3 vs 911 at 2)
        #  pw: [128,512] x2 = 2 banks (q/g1/V projections, PV
        #      accumulators, out-proj tiles)
        sc_pool = ctx.enter_context(
            tc.tile_pool(name="sc_psum", bufs=3, space="PSUM"))
        pw_pool = ctx.enter_context(
            tc.tile_pool(name="pw_psum", bufs=2, space="PSUM"))
        sm_pool = ctx.enter_context(tc.tile_pool(name="sm", bufs=1))
        nrm_pool = ctx.enter_context(tc.tile_pool(name="nrm", bufs=4))
        fo_pool = ctx.enter_context(tc.tile_pool(name="fo", bufs=2))

        e01_t = {}       # qb -> expt tile h0/h1: [128, NKC, 2, 512]
        e2_t = {}        # qb -> expt tile h2:    [128, NKC, 512]
        pvps = {}        # h -> current PV PSUM tile

        # ---- k/q g0 projection for one 512-col block into psum ----
        def emit_proj_block(ps, name, qb):
            x_sb = ins_sb[name + "t"]
            for c in range(NC_):
                nc.tensor.matmul(
                    ps, lhsT=w_sb["w" + name][:, c, 0:128],
                    rhs=x_sb[:, c, ts(qb, 512)],
                    start=(c == 0), stop=(c == NC_ - 1))

        def evac_proj(ps, name, qb):
            nc.vector.tensor_scalar_add(
                out=qk0[name][:, ts(qb, 512)], in0=ps,
                scalar1=bias_sb["b" + name][0])

        # ---- units: closures emitted by the pipeline zipper ----
        def u_qproj(qb):
            def emit():
                ps = pw_pool.tile([128, 512], f32, tag="pw", name="pw")
                emit_proj_block(ps, "q", qb)
                evac_proj(ps, "q", qb)
            return emit

        def u_g1proj(qb):
            # head 2 q/k: M=64 pair on PE col groups 0-1 / 2-3
            def emit():
                ps = pw_pool.tile([128, 512], f32, tag="pw", name="pw")
                for c in range(NC_):
                    nc.tensor.matmul(
                        ps[0:64, :], lhsT=w_sb["wq"][:, c, 128:192],
                        rhs=ins_sb["qt"][:, c, ts(qb, 512)],
                        start=(c == 0), stop=(c == NC_ - 1),
                        tile_position=(0, 0))
                    nc.tensor.matmul(
                        ps[64:128, :], lhsT=w_sb["wk"][:, c, 128:192],
                        rhs=ins_sb["kt"][:, c, ts(qb, 512)],
                        start=(c == 0), stop=(c == NC_ - 1),
                        tile_position=(0, 64))
                nc.vector.tensor_scalar_add(
                    out=qk1["q"][0:64, ts(qb, 512)], in0=ps[0:64, :],
                    scalar1=bias_sb["bq"][1])
                nc.vector.tensor_scalar_add(
                    out=qk1["k"][0:64, ts(qb, 512)], in0=ps[64:128, :],
                    scalar1=bias_sb["bk"][1])
            return emit

        def u_g1dup():
            # duplicate head-2 q/k onto partitions 64..127 so its score
            # matmuls can run as row-tile pairs too
            def emit():
                nc.sync.dma_start(out=qk1["q"][64:128, :],
                                  in_=qk1["q"][0:64, :])
                nc.sync.dma_start(out=qk1["k"][64:128, :],
                                  in_=qk1["k"][0:64, :])
            return emit

        def u_vproj(st):
            def emit():
                ps = pw_pool.tile([128, 512], f32, tag="pw", name="pw")
                for c in range(NC_):
                    nc.tensor.matmul(
                        ps[:, 0:HD], lhsT=ins_sb["vt"][:, c, ts(st, 128)],
                        rhs=w_sb["wv"][:, c, :],
                        start=(c == 0), stop=(c == NC_ - 1))
                for h in range(HPC):
                    nc.vector.tensor_add(
                        v_sb[:, st, h * VW:h * VW + DK],
                        ps[:, ts(h, 64)], bv_bc[:, ts(h, 64)])
            return emit

        def u_sc01(qb, c):
            # heads 0+1, kv chunk c, as an adjacent row-tile pair into one
            # [128,2,512] tile; one 1024-elem exp
            def emit():
                if c == 0:
                    e01_t[qb] = sm_pool.tile([128, NKC, 2, 512], bf16,
                                             tag="e01", name="e01")
                ps = sc_pool.tile([128, 2, 512], f32, tag="sc", name="sc")
                for h in (0, 1):
                    nc.tensor.matmul(
                        ps[:, h, :],
                        lhsT=qk0["k"][ts(h, 64), ts(c, 128)],
                        rhs=qk0["q"][ts(h, 64), ts(qb, 512)],
                        start=True, stop=True)
                nc.scalar.activation(
                    out=e01_t[qb][:, c, :, :], in_=ps[:, 0:2, :],
                    func=mybir.ActivationFunctionType.Exp, scale=SCALE)
            return emit

        def u_sc2(qb, p):
            # head 2, kv chunks {2p, 2p+1} as a row-tile pair (second
            # operand pair comes from the duplicated partitions 64..127)
            def emit():
                if p == 0:
                    e2_t[qb] = sm_pool.tile([128, NKC, 512], bf16,
                                            tag="e2", name="e2", bufs=2)
                ps = sc_pool.tile([128, 2, 512], f32, tag="sc", name="sc")
                for j in (0, 1):
                    nc.tensor.matmul(
                        ps[:, j, :],
                        lhsT=qk1["k"][ts(j, 64), ts(2 * p + j, 128)],
                        rhs=qk1["q"][ts(j, 64), ts(qb, 512)],
                        start=True, stop=True)
                nc.scalar.activation(
                    out=e2_t[qb][:, 2 * p:2 * p + 2, :], in_=ps[:, 0:2, :],
                    func=mybir.ActivationFunctionType.Exp, scale=SCALE)
            return emit

        def u_pv(h, qb, half):
            # PV accumulation, kv chunks [8*half, 8*half+8)
            def emit():
                if half == 0:
                    pvps[h] = pw_pool.tile([128, 512], f32, tag="pw",
                                           name="pw")
                ps = pvps[h]
                for kc in range(8 * half, 8 * half + 8):
                    rhs = (e01_t[qb][:, kc, h, :] if h < 2
                           else e2_t[qb][:, kc, :])
                    nc.tensor.matmul(
                        ps[0:DK + 1, :],
                        lhsT=v_sb[:, kc, h * VW:h * VW + DK + 1],
                        rhs=rhs,
                        start=(kc == 0), stop=(kc == NKC - 1))
            return emit

        def u_norm(h, qb):
            # reciprocal of the ones-row -> SBUF, partition-broadcast on
            # the idle gpsimd engine, then normalize on DVE.
            def emit():
                ps = pvps[h]
                recip = nrm_pool.tile([1, 512], bf16, tag="recip",
                                      name="recip")
                with nc.allow_low_precision(reason="softmax recip in bf16"):
                    nc.vector.reciprocal(recip, ps[DK:DK + 1, :])
                rbc = nrm_pool.tile([64, 512], bf16, tag="rbc", name="rbc")
                nc.gpsimd.partition_broadcast(rbc, recip)
                dst = (outT0[ts(h, 64), ts(qb, 512)] if h < 2
                       else outT1[0:64, ts(qb, 512)])
                nc.vector.tensor_mul(dst, ps[0:DK, :], rbc)
            return emit

        ot_tile = {}

        def u_oproj(qb, sq):
            # output projection for q sub-tile qb*4+sq, N split 512+256
            def emit():
                if sq == 0:
                    ot_tile[qb] = fo_pool.tile([128, 4, D], bf16, tag="ot",
                                               name="ot")
                qt_ = qb * 4 + sq
                for noff, nsz in ((0, 512), (512, 256)):
                    ps = pw_pool.tile([128, 512], f32, tag="pw", name="pw")
                    nc.tensor.matmul(
                        ps[:, 0:nsz], lhsT=outT0[:, ts(qt_, 128)],
                        rhs=wo0[:, noff:noff + nsz], start=True, stop=False)
                    nc.tensor.matmul(
                        ps[:, 0:nsz], lhsT=outT1[:, ts(qt_, 128)],
                        rhs=wo1[:, noff:noff + nsz], start=False, stop=True)
                    nc.vector.tensor_copy(
                        out=ot_tile[qb][:, sq, noff:noff + nsz],
                        in_=ps[:, 0:nsz])
            return emit

        def u_out(qb):
            def emit():
                nc.sync.dma_start(
                    out=out_part[:, :].rearrange(
                        "(g t p) d -> g p t d", p=128, t=4)[qb],
                    in_=ot_tile[qb])
                if collective:
                    nc.gpsimd.collective_compute(
                        "ReduceScatter", mybir.AluOpType.add,
                        replica_groups=RGROUPS,
                        ins=[out_part[ts(qb, 512), :]],
                        outs=[out_rs[ts(qb, 128), :]])
                nc.sync.dma_start(out=out_ext[ts(qb, 128), :],
                                  in_=(out_rs if collective
                                       else out_part)[ts(qb, 128), :])
            return emit

        def zip_emit(primary, secondary):
            """Interleave unit lists: primary paces (ACT-gated score
            rounds), secondary fills PE slack in mode-pure chunks."""
            n = max(len(primary), len(secondary))
            for i in range(n):
                if i < len(primary):
                    primary[i]()
                if i < len(secondary):
                    secondary[i]()

        # ---- prologue: k projection (both k-qb pairs share one sc tile
        # via the 2-chunk slots), then q block 0 ----
        for pair in (0, 1):
            ps = sc_pool.tile([128, 2, 512], f32, tag="sc", name="sc")
            for j in (0, 1):
                qb_ = 2 * pair + j
                emit_proj_block(ps[:, j, :], "k", qb_)
                evac_proj(ps[:, j, :], "k", qb_)
        ps = pw_pool.tile([128, 512], f32, tag="pw", name="pw")
        emit_proj_block(ps, "q", 0)
        evac_proj(ps, "q", 0)

        # ---- pipeline over q blocks ----
        # iteration qb emits its own score rounds (h2 first, then h0/h1;
        # qb0: h0/h1 first since head 2 projection happens inside qb0's
        # secondary list), with the previous block's PV/norm/out-proj
        # (and qb's own h2 PV) zipped between rounds in mode-pure chunks.
        for it in range(NQB + 1):
            qb, pq = it, it - 1
            if it == 0:
                rounds = ([u_sc01(0, c) for c in range(NKC)] +
                          [u_sc2(0, p) for p in range(NKC // 2)])
                sec = ([u_qproj(1), u_qproj(2), u_qproj(3)] +
                       [u_g1proj(qb_) for qb_ in range(NQB)] +
                       [u_g1dup()] +
                       [u_vproj(st) for st in range(NKC)])
            elif it < NQB:
                rounds = ([u_sc2(qb, p) for p in range(NKC // 2)] +
                          [u_sc01(qb, c) for c in range(NKC)])
                # pv2(pq) must come first: the new h2 exps reuse its tile
                sec = [u_pv(2, pq, 0), u_pv(2, pq, 1), u_norm(2, pq),
                       u_pv(0, pq, 0), u_pv(0, pq, 1), u_norm(0, pq),
                       u_pv(1, pq, 0), u_pv(1, pq, 1), u_norm(1, pq),
                       u_oproj(pq, 0), u_oproj(pq, 1),
                       u_oproj(pq, 2), u_oproj(pq, 3), u_out(pq)]
                if it == NQB - 1:
                    # drain qb3's h2 inside its own iteration to shorten
                    # the epilogue
                    sec += [u_pv(2, qb, 0), u_pv(2, qb, 1), u_norm(2, qb)]
            else:
                rounds = []
                sec = [u_pv(0, pq, 0), u_pv(0, pq, 1), u_norm(0, pq),
                       u_pv(1, pq, 0), u_pv(1, pq, 1), u_norm(1, pq),
                       u_oproj(pq, 0), u_oproj(pq, 1),
                       u_oproj(pq, 2), u_oproj(pq, 3), u_out(pq)]
            zip_emit(rounds, sec)

    nc.compile()
    return nc


def _get_compiled():
    global _compiled
    if _compiled is None:
        _compiled = _build()
    return _compiled


def make_in_maps(q, k, v, Wq, bq, Wk, bk, Wv, bv, Wo, bo):
    bf = ml_dtypes.bfloat16
    in_maps = []
    for c in range(NCORES):
        b = c // GROUP
        g = c % GROUP
        cols = slice(g * HD, (g + 1) * HD)   # head-feature columns
        wo_aug = np.empty((HD + 1, D), np.float32)
        wo_aug[:HD] = Wo.T[cols.start:cols.stop, :]
        wo_aug[HD] = bo / GROUP              # summed GROUP times by the RS
        in_maps.append({
            "qt": np.ascontiguousarray(q[b].T).astype(bf),
            "kt": np.ascontiguousarray(k[b].T).astype(bf),
            "vt": np.ascontiguousarray(v[b].T).astype(bf),
            "wq": np.ascontiguousarray(Wq.T[:, cols]).astype(bf),
            "wk": np.ascontiguousarray(Wk.T[:, cols]).astype(bf),
            "wv": np.ascontiguousarray(Wv.T[:, cols]).astype(bf),
            "wo": wo_aug.astype(bf),
            "bq": np.ascontiguousarray(bq[cols].reshape(HD, 1)).astype(np.float32),
            "bk": np.ascontiguousarray(bk[cols].reshape(HD, 1)).astype(np.float32),
            "bv": np.ascontiguousarray(bv[cols].reshape(1, HD)).astype(np.float32),
        })
    return in_maps


def kernel(q, k, v, Wq, bq, Wk, bk, Wv, bv, Wo, bo):
    from concourse.bass_utils import run_bass_kernel_spmd

    q = np.asarray(q, np.float32)
    k = np.asarray(k, np.float32)
    v = np.asarray(v, np.float32)
    nc = _get_compiled()
    in_maps = make_in_maps(q, k, v,
                           np.asarray(Wq, np.float32), np.asarray(bq, np.float32),
                           np.asarray(Wk, np.float32), np.asarray(bk, np.float32),
                           np.asarray(Wv, np.float32), np.asarray(bv, np.float32),
                           np.asarray(Wo, np.float32), np.asarray(bo, np.float32))
    res = run_bass_kernel_spmd(nc, in_maps, list(range(NCORES))).results
    out = np.empty((B, S, D), np.float32)
    for c in range(NCORES):
        b = c // GROUP
        j = c % GROUP
        # chunked reduce-scatter: chunk g of core (b, j) holds batch-b
        # rows [512*g + 128*j, 512*g + 128*j + 128)
        chunks = res[c]["out"].reshape(GROUP, 128, D)
        for g in range(GROUP):
            out[b, 512 * g + 128 * j:512 * g + 128 * j + 128, :] = chunks[g]
    return out
